# revision 1
# baseline (speedup 1.0000x reference)
"""Trainium2 Bass kernel for banded local attention.

Reference computation (B=2, S=2048, D=512, H=8, dh=64, local_range=7):
  q = hs @ Wq, k = hs @ Wk (per-head slices)
  scores = q k^T / sqrt(dh); w = softmax(scores) * band; w /= sum(w) + 1e-6
  ctx = w @ hs                                  -> [B, H, S, D]

Since w is re-normalized over the band, softmax(scores)*band/sum ==
band-limited softmax up to the tiny 1e-6*Z correction (~1e-4 relative),
so we only ever compute the 15-diagonal band of scores.

Sharding: 16 (b, h) pairs over 8 cores -> core c handles batch c//4 and
heads 2*(c%4), 2*(c%4)+1. Each core computes its two heads' projections
and banded attention; host gathers [2, S, D] per core into [B, H, S, D].
"""

import numpy as np
import ml_dtypes

BF = ml_dtypes.bfloat16
S, D, H, DH = 2048, 512, 8, 64
NCORES = 8
NT = S // 128  # 16 row tiles
KT = D // 128  # 4 contraction tiles
MASKW = 160
NEG = -10000.0  # exp(x + NEG) == 0.0 in fp32 for any realistic score x

TRACE = False
LAST_RESULTS = None

_NC_CACHE = {}


def _build_nc():
    import concourse.bacc as bacc
    import concourse.mybir as mybir
    import concourse.tile as tile

    f32 = mybir.dt.float32
    bf16 = mybir.dt.bfloat16
    AF = mybir.ActivationFunctionType

    nc = bacc.Bacc("TRN2", target_bir_lowering=False, debug=False, num_devices=NCORES)

    hs = nc.dram_tensor("hs", [S, D], bf16, kind="ExternalInput").ap()
    hsd = nc.dram_tensor("hsd", [D, S], bf16, kind="ExternalInput").ap()
    wqk = nc.dram_tensor("wqk", [128, KT, 256], bf16, kind="ExternalInput").ap()
    # identity (cols 0:128) + band mask (cols 128:128+MASKW) packed so DMA
    # rows are >=512B (sub-512B descriptors pay 2x in the DMA engines)
    cmask = nc.dram_tensor("cmask", [128, 128 + MASKW], bf16, kind="ExternalInput").ap()
    out = nc.dram_tensor("out", [2, S, D], f32, kind="ExternalOutput").ap()
    # [S, 2, 512] view of out so one DMA writes both heads' rows for a tile
    out_r = out.rearrange("h s d -> s h d")

    with tile.TileContext(nc) as tc:
        with (
            tc.tile_pool(name="const", bufs=1) as cpool,
            tc.tile_pool(name="work", bufs=8) as wpool,
            tc.tile_pool(name="outp", bufs=4) as opool,
            tc.tile_pool(name="psum_band", bufs=2, space="PSUM") as pband,
            tc.tile_pool(name="psum_ctx", bufs=3, space="PSUM") as pctx_pool,
        ):
            # ---- constants / persistent tiles ----
            # small weights first so the first projection matmul isn't gated
            # behind the big transposed loads in the DMA queue; wq/wk are
            # pre-packed host-side to [p, t, q|k] so rows are 2KB-contiguous
            wqk_sb = cpool.tile([128, KT, 256], bf16)
            nc.sync.dma_start(out=wqk_sb, in_=wqk)
            hsT = cpool.tile([128, KT, S], bf16)  # hs transposed: [d%128, d//128, s]
            hsd_r = hsd.rearrange("(t p) s -> p t s", p=128)
            for ssl in (slice(0, 256), slice(256, 512), slice(512, 1024),
                        slice(1024, 1536), slice(1536, 2048)):
                nc.sync.dma_start(out=hsT[:, :, ssl], in_=hsd_r[:, :, ssl])
            cmask_sb = cpool.tile([128, 128 + MASKW], bf16)
            nc.sync.dma_start(out=cmask_sb, in_=cmask)
            id_sb = cmask_sb[:, 0:128]
            mask_sb = cmask_sb[:, 128:128 + MASKW]

            # Banded hs windows, partition-aligned per tile:
            #   slot t = hs rows [t*128-7, t*128+121)   (slot 0 = rows [0,128))
            # Tail rows for tile t (j in [t*128+121, t*128+135)) are the first
            # 14 partitions of slot t+1; t=0 and t=15 need small specials.
            w0all = cpool.tile([128, NT, 512], bf16)
            w1s0 = cpool.tile([7, 512], bf16)   # t=0 tail: hs rows [128, 135)
            nc.sync.dma_start(out=w1s0, in_=hs[128:135, :])
            nc.sync.dma_start(out=w0all[:, 0], in_=hs[0:128, :])
            nc.sync.dma_start(
                out=w0all[:, 1:4],
                in_=hs[121:121 + 3 * 128, :].rearrange("(t p) d -> p t d", p=128),
            )
            nc.sync.dma_start(
                out=w0all[:, 4:NT],
                in_=hs[121 + 3 * 128:121 + (NT - 1) * 128, :].rearrange("(t p) d -> p t d", p=128),
            )
            w1s15 = cpool.tile([7, 512], bf16)  # t=15 tail: hs rows [2041, 2048)
            nc.sync.dma_start(out=w1s15, in_=hs[2041:2048, :])

            # ---- projections: qT/kT = [128 (2 heads x 64), S] bf16 ----
            qT = cpool.tile([128, S], bf16)
            kTt = cpool.tile([128, S], bf16)

            def emit_proj_chunk(c):
                for half in range(2):
                    sl = slice(c * 512 + half * 256, c * 512 + (half + 1) * 256)
                    pq = pctx_pool.tile([128, 512], f32, tag="pctx")
                    for kt in range(KT):
                        nc.tensor.matmul(
                            pq[:, 0:256], wqk_sb[:, kt, 0:128], hsT[:, kt, sl],
                            start=(kt == 0), stop=(kt == KT - 1),
                        )
                    # fold 1/sqrt(dh) into q during PSUM eviction
                    nc.scalar.activation(qT[:, sl], pq[:, 0:256], AF.Copy, scale=1.0 / (DH ** 0.5))
                    pk = pctx_pool.tile([128, 512], f32, tag="pctx")
                    for kt in range(KT):
                        nc.tensor.matmul(
                            pk[:, 0:256], wqk_sb[:, kt, 128:256], hsT[:, kt, sl],
                            start=(kt == 0), stop=(kt == KT - 1),
                        )
                    nc.vector.tensor_copy(kTt[:, sl], pk[:, 0:256])

            # ---- banded attention per 128-row tile, both heads merged ----
            def emit_band_tile(t):
                i0 = t * 128
                jlo = max(i0 - 7, 0)
                jhi = min(i0 + 135, S)
                n = jhi - jlo        # 135 (first/last) or 142
                n1 = n - 128         # 7 or 14
                moff = 7 if t == 0 else 0
                if t == 0:
                    rhs1 = w1s0
                elif t == NT - 1:
                    rhs1 = w1s15
                else:
                    rhs1 = w0all[0:n1, t + 1]

                o2 = opool.tile([128, 2, 512], f32, tag="o")
                # scores + band mask in ONE psum accumulation group:
                # mask rows land via identity^T @ mask, scores accumulate on top
                psc2 = pband.tile([128, 284], f32, tag="psc", bufs=3)
                for h in range(2):
                    hp = slice(h * 64, (h + 1) * 64)
                    csl = slice(142 * h, 142 * h + n)
                    nc.tensor.matmul(
                        psc2[:, csl], id_sb, mask_sb[:, moff:moff + n],
                        start=(h == 0), stop=False,
                    )
                    nc.tensor.matmul(
                        psc2[:, csl], qT[hp, i0:i0 + 128], kTt[hp, jlo:jhi],
                        start=False, stop=(h == 1),
                    )
                pscv = psc2[:].rearrange("p (h m) -> p h m", h=2)[:, :, :n]
                E2 = wpool.tile([128, 284], bf16, tag="E")
                E2v = E2[:].rearrange("p (h m) -> p h m", h=2)[:, :, :n]
                nc.scalar.activation(E2v, pscv, AF.Exp)
                s2 = wpool.tile([128, 2], f32, tag="s")
                nc.vector.tensor_reduce(s2, E2v, axis=mybir.AxisListType.X, op=mybir.AluOpType.add)
                r2 = wpool.tile([128, 2], f32, tag="r")
                nc.vector.reciprocal(r2, s2)
                # transpose E to [j, i]: pt2 = [T0h0 | T0h1 | T1h0 | T1h1]
                pt2 = pband.tile([128, 512], bf16, tag="pt")
                nc.tensor.transpose(pt2[:, 0:128], E2[:, 0:128], id_sb)
                nc.tensor.transpose(pt2[:, 128:256], E2[:, 142:270], id_sb)
                nc.tensor.transpose(pt2[:n1, 256:384], E2[:, 128:n], id_sb)
                nc.tensor.transpose(pt2[:n1, 384:512], E2[:, 270:270 + n1], id_sb)
                ET2 = wpool.tile([128, 512], bf16, tag="ET")
                nc.scalar.copy(ET2[:, 0:256], pt2[:, 0:256])
                nc.vector.tensor_copy(ET2[:n1, 256:512], pt2[:n1, 256:512])
                # ctx[i, :] = sum_j E[j, i] * hs[j, :] per head; normalize on evict
                for h in range(2):
                    pctx = pctx_pool.tile([128, 512], f32, tag="pctx")
                    nc.tensor.matmul(pctx, ET2[:, 128 * h:128 * h + 128], w0all[:, t],
                                     start=True, stop=False)
                    nc.tensor.matmul(pctx, ET2[:n1, 256 + 128 * h:256 + 128 * h + 128], rhs1,
                                     start=False, stop=True)
                    if h == 0:
                        nc.vector.tensor_scalar_mul(o2[:, 0], pctx, r2[:, 0:1])
                    else:
                        nc.scalar.activation(o2[:, 1], pctx, AF.Copy, scale=r2[:, 1:2])
                    nc.sync.dma_start(out=out[h, i0:i0 + 128, :], in_=o2[:, h])


            # interleave: after projection chunk c, band tiles needing only
            # chunks <= c are emitted so DVE/ACT streams don't serialize phases
            emit_proj_chunk(0)
            emit_proj_chunk(1)
            for t in range(0, 3):
                emit_band_tile(t)
            emit_proj_chunk(2)
            for t in range(3, 7):
                emit_band_tile(t)
            emit_proj_chunk(3)
            for t in range(7, 16):
                emit_band_tile(t)

    nc.compile()
    return nc


def _get_nc():
    if "nc" not in _NC_CACHE:
        _NC_CACHE["nc"] = _build_nc()
    return _NC_CACHE["nc"]


def kernel(hidden_states, Wq, Wk):
    global LAST_RESULTS
    from concourse import bass_utils

    B = hidden_states.shape[0]
    hs_bf = np.asarray(hidden_states).astype(BF)
    wq_bf = np.asarray(Wq).astype(BF)
    wk_bf = np.asarray(Wk).astype(BF)

    p = np.arange(128)[:, None]
    f = np.arange(MASKW)[None, :]
    maskb = np.where((f - p >= 0) & (f - p <= 14), 0.0, NEG).astype(BF)
    cmask = np.concatenate([np.eye(128, dtype=BF), maskb], axis=1)

    wqk_packed = []
    for c in range(NCORES):
        h0 = 2 * (c % 4)
        wqs = wq_bf[:, h0 * DH:(h0 + 2) * DH].reshape(KT, 128, 128)
        wks = wk_bf[:, h0 * DH:(h0 + 2) * DH].reshape(KT, 128, 128)
        packed = np.concatenate([wqs, wks], axis=2)       # [KT, 128(p), 256]
        wqk_packed.append(np.ascontiguousarray(packed.transpose(1, 0, 2)))

    in_maps = []
    for c in range(NCORES):
        b = c // 4
        h0 = 2 * (c % 4)
        in_maps.append({
            "hs": np.ascontiguousarray(hs_bf[b]),
            "hsd": np.ascontiguousarray(hs_bf[b].T),
            "wqk": wqk_packed[c],
            "cmask": cmask,
        })

    nc = _get_nc()
    res = bass_utils.run_bass_kernel_spmd(
        nc, in_maps, core_ids=list(range(NCORES)), trace=TRACE,
    )
    LAST_RESULTS = res

    out = np.empty((B, H, S, D), np.float32)
    for c in range(NCORES):
        b = c // 4
        h0 = 2 * (c % 4)
        out[b, h0:h0 + 2] = res.results[c]["out"]
    return out



# revision 4
# speedup vs baseline: 1.0103x; 1.0103x over previous
"""Trainium2 Bass kernel for banded local attention.

Reference computation (B=2, S=2048, D=512, H=8, dh=64, local_range=7):
  q = hs @ Wq, k = hs @ Wk (per-head slices)
  scores = q k^T / sqrt(dh); w = softmax(scores) * band; w /= sum(w) + 1e-6
  ctx = w @ hs                                  -> [B, H, S, D]

Since w is re-normalized over the band, softmax(scores)*band/sum ==
band-limited softmax up to the tiny 1e-6*Z correction (~1e-4 relative),
so we only ever compute the 15-diagonal band of scores.

Sharding: 16 (b, h) pairs over 8 cores -> core c handles batch c//4 and
heads 2*(c%4), 2*(c%4)+1. Each core computes its two heads' projections
and banded attention; host gathers [2, S, D] per core into [B, H, S, D].
"""

import numpy as np
import ml_dtypes

BF = ml_dtypes.bfloat16
S, D, H, DH = 2048, 512, 8, 64
NCORES = 8
NT = S // 128  # 16 row tiles
KT = D // 128  # 4 contraction tiles
MASKW = 160
NEG = -10000.0  # exp(x + NEG) == 0.0 in fp32 for any realistic score x

TRACE = False
LAST_RESULTS = None

_NC_CACHE = {}


def _build_nc():
    import concourse.bacc as bacc
    import concourse.mybir as mybir
    import concourse.tile as tile

    f32 = mybir.dt.float32
    bf16 = mybir.dt.bfloat16
    AF = mybir.ActivationFunctionType

    nc = bacc.Bacc("TRN2", target_bir_lowering=False, debug=False, num_devices=NCORES)

    hs = nc.dram_tensor("hs", [S, D], bf16, kind="ExternalInput").ap()
    hsd = nc.dram_tensor("hsd", [D, S], bf16, kind="ExternalInput").ap()
    wqk = nc.dram_tensor("wqk", [128, KT, 256], bf16, kind="ExternalInput").ap()
    # identity (cols 0:128) + band mask (cols 128:128+MASKW) packed so DMA
    # rows are >=512B (sub-512B descriptors pay 2x in the DMA engines)
    cmask = nc.dram_tensor("cmask", [128, 128 + MASKW], bf16, kind="ExternalInput").ap()
    # bf16 output: halves the dominant output DMA; host casts back to f32
    out = nc.dram_tensor("out", [2, S, D], bf16, kind="ExternalOutput").ap()

    with tile.TileContext(nc) as tc:
        with (
            tc.tile_pool(name="const", bufs=1) as cpool,
            tc.tile_pool(name="work", bufs=8) as wpool,
            tc.tile_pool(name="outp", bufs=4) as opool,
            tc.tile_pool(name="psum_band", bufs=2, space="PSUM") as pband,
            tc.tile_pool(name="psum_ctx", bufs=3, space="PSUM") as pctx_pool,
        ):
            # ---- constants / persistent tiles ----
            # small weights first so the first projection matmul isn't gated
            # behind the big transposed loads in the DMA queue; wq/wk are
            # pre-packed host-side to [p, t, q|k] so rows are 2KB-contiguous
            wqk_sb = cpool.tile([128, KT, 256], bf16)
            nc.sync.dma_start(out=wqk_sb, in_=wqk)
            hsT = cpool.tile([128, KT, S], bf16)  # hs transposed: [d%128, d//128, s]
            hsd_r = hsd.rearrange("(t p) s -> p t s", p=128)
            for ssl in (slice(0, 256), slice(256, 512), slice(512, 1024),
                        slice(1024, 1536), slice(1536, 2048)):
                nc.sync.dma_start(out=hsT[:, :, ssl], in_=hsd_r[:, :, ssl])
            cmask_sb = cpool.tile([128, 128 + MASKW], bf16)
            nc.sync.dma_start(out=cmask_sb, in_=cmask)
            id_sb = cmask_sb[:, 0:128]
            mask_sb = cmask_sb[:, 128:128 + MASKW]

            # Banded hs windows, partition-aligned per tile:
            #   slot t = hs rows [t*128-7, t*128+121)   (slot 0 = rows [0,128))
            # Tail rows for tile t (j in [t*128+121, t*128+135)) are the first
            # 14 partitions of slot t+1; t=0 and t=15 need small specials.
            w0all = cpool.tile([128, NT, 512], bf16)
            w1s0 = cpool.tile([7, 512], bf16)   # t=0 tail: hs rows [128, 135)
            nc.sync.dma_start(out=w1s0, in_=hs[128:135, :])
            nc.sync.dma_start(out=w0all[:, 0], in_=hs[0:128, :])
            nc.sync.dma_start(
                out=w0all[:, 1:4],
                in_=hs[121:121 + 3 * 128, :].rearrange("(t p) d -> p t d", p=128),
            )
            nc.sync.dma_start(
                out=w0all[:, 4:NT],
                in_=hs[121 + 3 * 128:121 + (NT - 1) * 128, :].rearrange("(t p) d -> p t d", p=128),
            )
            w1s15 = cpool.tile([7, 512], bf16)  # t=15 tail: hs rows [2041, 2048)
            nc.sync.dma_start(out=w1s15, in_=hs[2041:2048, :])

            # ---- projections: qT/kT = [128 (2 heads x 64), S] bf16 ----
            qT = cpool.tile([128, S], bf16)
            kTt = cpool.tile([128, S], bf16)

            def emit_proj_chunk(c):
                for half in range(2):
                    sl = slice(c * 512 + half * 256, c * 512 + (half + 1) * 256)
                    pq = pctx_pool.tile([128, 512], f32, tag="pctx")
                    for kt in range(KT):
                        nc.tensor.matmul(
                            pq[:, 0:256], wqk_sb[:, kt, 0:128], hsT[:, kt, sl],
                            start=(kt == 0), stop=(kt == KT - 1),
                        )
                    # fold 1/sqrt(dh) into q during PSUM eviction
                    nc.scalar.activation(qT[:, sl], pq[:, 0:256], AF.Copy, scale=1.0 / (DH ** 0.5))
                    pk = pctx_pool.tile([128, 512], f32, tag="pctx")
                    for kt in range(KT):
                        nc.tensor.matmul(
                            pk[:, 0:256], wqk_sb[:, kt, 128:256], hsT[:, kt, sl],
                            start=(kt == 0), stop=(kt == KT - 1),
                        )
                    nc.vector.tensor_copy(kTt[:, sl], pk[:, 0:256])

            # ---- banded attention per 128-row tile, both heads merged ----
            def emit_band_tile(t):
                i0 = t * 128
                jlo = max(i0 - 7, 0)
                jhi = min(i0 + 135, S)
                n = jhi - jlo        # 135 (first/last) or 142
                n1 = n - 128         # 7 or 14
                moff = 7 if t == 0 else 0
                if t == 0:
                    rhs1 = w1s0
                elif t == NT - 1:
                    rhs1 = w1s15
                else:
                    rhs1 = w0all[0:n1, t + 1]

                o2 = opool.tile([128, 2, 512], bf16, tag="o")
                # scores + band mask in ONE psum accumulation group:
                # mask rows land via identity^T @ mask, scores accumulate on top
                psc2 = pband.tile([128, 284], f32, tag="psc", bufs=3)
                for h in range(2):
                    hp = slice(h * 64, (h + 1) * 64)
                    csl = slice(142 * h, 142 * h + n)
                    nc.tensor.matmul(
                        psc2[:, csl], id_sb, mask_sb[:, moff:moff + n],
                        start=(h == 0), stop=False,
                    )
                    nc.tensor.matmul(
                        psc2[:, csl], qT[hp, i0:i0 + 128], kTt[hp, jlo:jhi],
                        start=False, stop=(h == 1),
                    )
                pscv = psc2[:].rearrange("p (h m) -> p h m", h=2)[:, :, :n]
                E2 = wpool.tile([128, 284], bf16, tag="E")
                E2v = E2[:].rearrange("p (h m) -> p h m", h=2)[:, :, :n]
                nc.scalar.activation(E2v, pscv, AF.Exp)
                s2 = wpool.tile([128, 2], f32, tag="s")
                nc.vector.tensor_reduce(s2, E2v, axis=mybir.AxisListType.X, op=mybir.AluOpType.add)
                r2 = wpool.tile([128, 2], f32, tag="r")
                nc.vector.reciprocal(r2, s2)
                # transpose E to [j, i]: pt2 = [T0h0 | T0h1 | T1h0 | T1h1]
                pt2 = pband.tile([128, 512], bf16, tag="pt")
                nc.tensor.transpose(pt2[:, 0:128], E2[:, 0:128], id_sb)
                nc.tensor.transpose(pt2[:, 128:256], E2[:, 142:270], id_sb)
                nc.tensor.transpose(pt2[:n1, 256:384], E2[:, 128:n], id_sb)
                nc.tensor.transpose(pt2[:n1, 384:512], E2[:, 270:270 + n1], id_sb)
                ET2 = wpool.tile([128, 512], bf16, tag="ET")
                nc.scalar.copy(ET2[:, 0:256], pt2[:, 0:256])
                nc.vector.tensor_copy(ET2[:n1, 256:512], pt2[:n1, 256:512])
                # ctx[i, :] = sum_j E[j, i] * hs[j, :] per head; normalize on evict
                for h in range(2):
                    pctx = pctx_pool.tile([128, 512], f32, tag="pctx")
                    nc.tensor.matmul(pctx, ET2[:, 128 * h:128 * h + 128], w0all[:, t],
                                     start=True, stop=False)
                    nc.tensor.matmul(pctx, ET2[:n1, 256 + 128 * h:256 + 128 * h + 128], rhs1,
                                     start=False, stop=True)
                    if h == 0:
                        nc.vector.tensor_scalar_mul(o2[:, 0], pctx, r2[:, 0:1])
                    else:
                        nc.scalar.activation(o2[:, 1], pctx, AF.Copy, scale=r2[:, 1:2])
                    nc.sync.dma_start(out=out[h, i0:i0 + 128, :], in_=o2[:, h])


            # interleave: after projection chunk c, band tiles needing only
            # chunks <= c are emitted so DVE/ACT streams don't serialize phases
            emit_proj_chunk(0)
            emit_proj_chunk(1)
            for t in range(0, 3):
                emit_band_tile(t)
            emit_proj_chunk(2)
            for t in range(3, 7):
                emit_band_tile(t)
            emit_proj_chunk(3)
            for t in range(7, 16):
                emit_band_tile(t)

    nc.compile()
    return nc


def _get_nc():
    if "nc" not in _NC_CACHE:
        _NC_CACHE["nc"] = _build_nc()
    return _NC_CACHE["nc"]


def kernel(hidden_states, Wq, Wk):
    global LAST_RESULTS
    from concourse import bass_utils

    B = hidden_states.shape[0]
    hs_bf = np.asarray(hidden_states).astype(BF)
    wq_bf = np.asarray(Wq).astype(BF)
    wk_bf = np.asarray(Wk).astype(BF)

    p = np.arange(128)[:, None]
    f = np.arange(MASKW)[None, :]
    maskb = np.where((f - p >= 0) & (f - p <= 14), 0.0, NEG).astype(BF)
    cmask = np.concatenate([np.eye(128, dtype=BF), maskb], axis=1)

    wqk_packed = []
    for c in range(NCORES):
        h0 = 2 * (c % 4)
        wqs = wq_bf[:, h0 * DH:(h0 + 2) * DH].reshape(KT, 128, 128)
        wks = wk_bf[:, h0 * DH:(h0 + 2) * DH].reshape(KT, 128, 128)
        packed = np.concatenate([wqs, wks], axis=2)       # [KT, 128(p), 256]
        wqk_packed.append(np.ascontiguousarray(packed.transpose(1, 0, 2)))

    in_maps = []
    for c in range(NCORES):
        b = c // 4
        h0 = 2 * (c % 4)
        in_maps.append({
            "hs": np.ascontiguousarray(hs_bf[b]),
            "hsd": np.ascontiguousarray(hs_bf[b].T),
            "wqk": wqk_packed[c],
            "cmask": cmask,
        })

    nc = _get_nc()
    res = bass_utils.run_bass_kernel_spmd(
        nc, in_maps, core_ids=list(range(NCORES)), trace=TRACE,
    )
    LAST_RESULTS = res

    out = np.empty((B, H, S, D), np.float32)
    for c in range(NCORES):
        b = c // 4
        h0 = 2 * (c % 4)
        out[b, h0:h0 + 2] = res.results[c]["out"].astype(np.float32)
    return out

